# revision 49
# baseline (speedup 1.0000x reference)
"""MoE layer (8 experts, top-2) on 8 TRN2 NeuronCores.

Strategy (expert-parallel with pairwise tensor-split, fp8 DoubleRow FFN):
  - Host computes the router exactly (fp32 numpy), does the top-2
    dispatch and ships the per-token combine weight, so the device does
    only the expert FFN.
  - Experts are sorted by load and split hot/cold; pair i = (hot_i,
    cold_i) is served by cores (2i, 2i+1), each holding one F-half of
    BOTH experts' weights. Both cores process the pair's full token
    list (segment A = hot tokens padded to S0, segment B = cold tokens
    padded to S1, S0/S1 shared across pairs so the SPMD program is
    uniform); the host adds the two half-F partial outputs. This costs
    (S0+S1)/2 full-F token-equivalents per core instead of S0 — load
    balancing that cuts PE time ~6%.
  - FFN runs on the PE in fp8-e4m3 DoubleRow mode (two 128-row k-tiles
    per instruction) with full error compensation: every operand is
    split into hi + lo fp8 parts (lo = residual of the hi quantization)
    and each matmul accumulates three passes in one PSUM group:
        hi@hi + lo@hi + hi@lo    (the lo@lo term is negligible)
    Weight tensors are pre-scaled by 256 on the host so every pass
    lands at the same power-of-2 scale; the 1/256 is folded into the
    gelu scale (mm1) and the combine weight (mm2).
  - Output f-blocks are processed in pairs sharing one [128, 2, 256]
    PSUM bank so ACT/DVE/DMA instruction counts stay half of PE's.
  - h = gelu(x @ w1 + b1) is written twice by the scalar engine (fp8 hi
    + f32), the DVE derives the fp8 lo residual.
  - The two head chunks' mm1s interleave by f-block so the PE covers
    the w1 DMA stream with no idle.
"""

from contextlib import ExitStack

import ml_dtypes
import numpy as np

P = 128
B, S, H, F, E = 2, 2048, 1024, 4096, 8
T = B * S            # 4096 tokens
FH = F // 2          # 2048 per-core F half
J = H // 256         # 4  mm1 k-tile pairs
G = FH // 256        # 8  mm2 k-tile pairs
FB = FH // P         # 16 mm1 output f-blocks
HB = H // 256        # 4  mm2 output h-blocks
CK = 256             # token chunk

fp8 = ml_dtypes.float8_e4m3fn

_CACHE = {}


def _chunks(S0, S1, L0, L1):
    """[(offset_in_C, csz, seg)]: 256-token chunks per segment, with the
    last chunk trimmed to the segment's actual max load (L) — matmul
    cost is proportional to the moving width, so tokens between L and
    the 128-padded capacity S are never computed. Partial chunks go
    last (smallest at the very end) so the end-of-program output drain
    trails the narrowest possible tile."""
    full, partial = [], []
    for seg, (base, load) in enumerate([(0, L0), (S0, L1)]):
        t0 = 0
        while t0 < load:
            csz = min(CK, load - t0)
            (full if csz == CK else partial).append((base + t0, csz, seg))
            t0 += csz
    partial.sort(key=lambda t: -t[1])
    return full + partial


def _build_nc(S0, S1, L0, L1, fuse1, fuse2):
    import concourse.mybir as mybir
    import concourse.tile as tile
    from concourse import bacc

    dt = mybir.dt
    AF = mybir.ActivationFunctionType
    ALU = mybir.AluOpType
    PM = mybir.MatmulPerfMode

    C = S0 + S1
    chunks = _chunks(S0, S1, L0, L1)
    NCT = len(chunks)
    TTS = C // P                     # token tiles

    nc = bacc.Bacc(
        "TRN2", target_bir_lowering=False, debug=False, num_devices=E)

    xh = nc.declare_dram_parameter("xh", [P, NCT * 2048], dt.float8e4, isOutput=False)
    xl = nc.declare_dram_parameter("xl", [P, NCT * 2048], dt.float8e4, isOutput=False)
    w1p = {}
    w2p = {}
    HBK = H // P                     # 8 mm2 output h-blocks of 128
    for s in "ab":
        w1p[s] = [nc.declare_dram_parameter(f"w1{s}{t}", [P, FB * 8 * P],
                                            dt.float8e4, isOutput=False)
                  for t in "hl"]
        w2p[s] = [nc.declare_dram_parameter(f"w2{s}{t}", [P, HBK * G * 2 * P],
                                            dt.float8e4, isOutput=False)
                  for t in "hl"]
    b1d = nc.declare_dram_parameter("b1d", [P, 2 * FB], dt.float32, isOutput=False)
    b2c = nc.declare_dram_parameter("b2c", [P, 2 * HBK], dt.float32, isOutput=False)
    wdv = nc.declare_dram_parameter("wdv", [P, C], dt.float32, isOutput=False)
    yc = nc.declare_dram_parameter("yc", [H, C], dt.float32, isOutput=True)

    xh_r = xh.rearrange("p (c j i t) -> p c j i t", c=NCT, j=J, i=2)
    xl_r = xl.rearrange("p (c j i t) -> p c j i t", c=NCT, j=J, i=2)
    w1r = {s: [a.rearrange("p (fb j i f) -> p fb j i f", fb=FB, j=J, i=2)
               for a in w1p[s]] for s in "ab"}
    w2r = {s: [a.rearrange("p (hb g i h) -> p hb g i h", hb=HBK, g=G, i=2)
               for a in w2p[s]] for s in "ab"}
    yc_r = yc.rearrange("(b p) t -> p b t", p=P)

    with ExitStack() as ctx:
        tc = ctx.enter_context(tile.TileContext(nc))
        const = ctx.enter_context(tc.tile_pool(name="const", bufs=1))
        # All DMAs issue on the single SP queue and a waiting DMA holds
        # the SP sequencer, so pools backing DMA-adjacent tiles must be
        # deep enough that no DMA ever waits on buffer reuse: x tiles
        # that do recycle buffers are loaded at the END of the input
        # stream, and the ob pool is deep enough that mm2 output muls
        # never wait for an output DMA to drain.
        xpool = ctx.enter_context(tc.tile_pool(name="xt", bufs=min(2 * NCT, 12)))
        h8pool = ctx.enter_context(tc.tile_pool(name="h8", bufs=2))
        hlpool = ctx.enter_context(tc.tile_pool(name="hl", bufs=2))
        gpool = ctx.enter_context(tc.tile_pool(name="g32", bufs=4))
        p1pool = ctx.enter_context(tc.tile_pool(name="p1", bufs=4, space="PSUM"))
        p2pool = ctx.enter_context(tc.tile_pool(name="p2", bufs=4, space="PSUM"))
        opool = ctx.enter_context(tc.tile_pool(name="ob", bufs=8))

        # ---- DMA schedule: head-chunk x first, then w1A in fine slices
        # (hi/lo interleaved), w2A, w1B, w2B, with remaining x chunks
        # threaded between. ----
        xh_s = [None] * NCT
        xl_s = [None] * NCT

        def load_x(c):
            xh_s[c] = xpool.tile([P, J, 2, CK], dt.float8e4, name="xt")
            xl_s[c] = xpool.tile([P, J, 2, CK], dt.float8e4, name="xt")
            nc.sync.dma_start(xh_s[c][:], xh_r[:, c])
            nc.sync.dma_start(xl_s[c][:], xl_r[:, c])

        b1_s = const.tile([P, 2, FB], dt.float32)
        wdv_s = const.tile([P, C], dt.float32)
        w1_s = {}
        w2_s = {}
        for s in "ab":
            w1_s[s] = [const.tile([P, FB, J, 2, P], dt.float8e4, name=f"w1{s}{t}")
                       for t in "hl"]
            w2_s[s] = [const.tile([P, HBK, G, 2, P], dt.float8e4, name=f"w2{s}{t}")
                       for t in "hl"]

        # PE p-state warmup: dummy DoubleRow matmuls on a zeroed tile
        # burn the cost model's clock ramp (~3us of accumulated busy
        # before full speed) during the otherwise-idle head DMA wait.
        wut = const.tile([P, 2, 256], dt.float8e4)
        nc.vector.memset(wut[:], 0)
        for i in range(32):
            pw = p1pool.tile([P, 2, CK], dt.float32, name="p1")
            nc.tensor.matmul(
                pw[:, 0], wut[:, :, :P], wut[:], start=True, stop=True,
                perf_mode=PM.DoubleRow)

        # b1 goes after the PE-critical x1/w1-s1 loads: only the ACT
        # engine (which has huge slack) waits on it.
        load_x(0)
        for si, (fb0, nfb) in enumerate([(0, 2), (2, 2), (4, 4), (8, 4), (12, 4)]):
            sl = slice(fb0, fb0 + nfb)
            if si == 1 and NCT > 1:
                load_x(1)
            nc.sync.dma_start(w1_s["a"][0][:, sl], w1r["a"][0][:, sl])
            nc.sync.dma_start(w1_s["a"][1][:, sl], w1r["a"][1][:, sl])
            if si == 1:
                nc.sync.dma_start(b1_s[:], b1d.rearrange("p (s f) -> p s f", s=2))
        nc.sync.dma_start(wdv_s[:], wdv[:])
        b2c_s = None
        if not fuse2:
            b2c_s = const.tile([P, 2, HBK], dt.float32)
        # x chunks that get fresh buffers interleave with the weight
        # stream; the tail chunks (recycled buffers, whose DMA waits for
        # the earlier reader) go last so the wait blocks nothing.
        nfresh = min(2 * NCT, 12) // 2
        nxt = 2
        for hb in range(0, HBK, 2):
            sl = slice(hb, hb + 2)
            nc.sync.dma_start(w2_s["a"][0][:, sl], w2r["a"][0][:, sl])
            nc.sync.dma_start(w2_s["a"][1][:, sl], w2r["a"][1][:, sl])
            if hb == 0 and not fuse2:
                nc.sync.dma_start(b2c_s[:], b2c.rearrange("p (s h) -> p s h", s=2))
            if nxt < nfresh:
                load_x(nxt)
                nxt += 1
        for fb0 in range(0, FB, 4):
            sl = slice(fb0, fb0 + 4)
            nc.sync.dma_start(w1_s["b"][0][:, sl], w1r["b"][0][:, sl])
            nc.sync.dma_start(w1_s["b"][1][:, sl], w1r["b"][1][:, sl])
            if nxt < nfresh:
                load_x(nxt)
                nxt += 1
        for hb in range(0, HBK, 2):
            sl = slice(hb, hb + 2)
            nc.sync.dma_start(w2_s["b"][0][:, sl], w2r["b"][0][:, sl])
            nc.sync.dma_start(w2_s["b"][1][:, sl], w2r["b"][1][:, sl])
            if nxt < nfresh:
                load_x(nxt)
                nxt += 1
        while nxt < NCT:
            load_x(nxt)
            nxt += 1

        hs = [None] * NCT

        def alloc_h(c):
            h8 = h8pool.tile([P, G, 2, CK], dt.float8e4, name="h8")
            hl = hlpool.tile([P, G, 2, CK], dt.float8e4, name="hl")
            hs[c] = (h8, hl)

        def emit_mm1_group(c, fbp):
            off, csz, seg = chunks[c]
            sk = "ab"[seg]
            w1hs, w1ls = w1_s[sk]
            xht, xlt = xh_s[c], xl_s[c]
            h8, hl = hs[c]
            ps = p1pool.tile([P, 2, CK], dt.float32, name="p1")
            for half in range(2):
                fb = 2 * fbp + half
                reg = ps[:, half, :csz]
                for j in range(J):
                    nc.tensor.matmul(
                        reg, w1hs[:, fb, j], xht[:, j, :, :csz],
                        start=(j == 0), stop=False, perf_mode=PM.DoubleRow)
                for j in range(J):
                    nc.tensor.matmul(
                        reg, w1hs[:, fb, j], xlt[:, j, :, :csz],
                        start=False, stop=False, perf_mode=PM.DoubleRow)
                for j in range(J):
                    nc.tensor.matmul(
                        reg, w1ls[:, fb, j], xht[:, j, :, :csz],
                        start=False, stop=(j == J - 1), perf_mode=PM.DoubleRow)
            g32 = gpool.tile([P, 2, CK], dt.float32, name="g32")
            h8v = h8[:, fbp, :, :csz]
            if fuse1:
                nc.scalar.activation(
                    g32[:, :, :csz], ps[:, :, :csz], AF.Gelu,
                    bias=0.0, scale=1.0 / 256)
                nc.scalar.activation(
                    h8v, ps[:, :, :csz], AF.Gelu, bias=0.0, scale=1.0 / 256)
            else:
                for half in range(2):
                    fb = 2 * fbp + half
                    nc.scalar.activation(
                        g32[:, half, :csz], ps[:, half, :csz], AF.Gelu,
                        bias=b1_s[:, seg, fb:fb + 1], scale=1.0 / 256)
                    nc.scalar.activation(
                        h8[:, fbp, half, :csz], ps[:, half, :csz], AF.Gelu,
                        bias=b1_s[:, seg, fb:fb + 1], scale=1.0 / 256)
            nc.vector.tensor_tensor(
                hl[:, fbp, :, :csz], g32[:, :, :csz], h8v, ALU.subtract)

        def emit_mm2(c):
            # Tokens ride the FREE dim (stationary w2, moving h), so mm2
            # cost is proportional to the chunk's actual token count and
            # the per-token combine weight is a plain elementwise mult.
            off, csz, seg = chunks[c]
            sk = "ab"[seg]
            w2hs, w2ls = w2_s[sk]
            h8, hl = hs[c]
            for hbp in range(HBK // 2):
                ps2 = p2pool.tile([P, 2, CK], dt.float32, name="p2")
                for half in range(2):
                    hb = 2 * hbp + half
                    reg = ps2[:, half, :csz]
                    for g in range(G):
                        nc.tensor.matmul(
                            reg, w2hs[:, hb, g], h8[:, g, :, :csz],
                            start=(g == 0), stop=False, perf_mode=PM.DoubleRow)
                    for g in range(G):
                        nc.tensor.matmul(
                            reg, w2hs[:, hb, g], hl[:, g, :, :csz],
                            start=False, stop=False, perf_mode=PM.DoubleRow)
                    for g in range(G):
                        nc.tensor.matmul(
                            reg, w2ls[:, hb, g], h8[:, g, :, :csz],
                            start=False, stop=(g == G - 1), perf_mode=PM.DoubleRow)
                ob = opool.tile([P, 2, CK], dt.float32, name="ob")
                for half in range(2):
                    hb = 2 * hbp + half
                    if fuse2:
                        nc.vector.tensor_tensor(
                            ob[:, half, :csz], ps2[:, half, :csz],
                            wdv_s[:, off:off + csz], ALU.mult)
                    else:
                        nc.vector.tensor_scalar_add(
                            ob[:, half, :csz], ps2[:, half, :csz],
                            b2c_s[:, seg, hb:hb + 1])
                        nc.vector.tensor_tensor(
                            ob[:, half, :csz], ob[:, half, :csz],
                            wdv_s[:, off:off + csz], ALU.mult)
                nc.sync.dma_start(
                    yc_r[:, 2 * hbp:2 * hbp + 2, off:off + csz],
                    ob[:, :, :csz])

        def emit_mm1(c):
            alloc_h(c)
            for fbp in range(FB // 2):
                emit_mm1_group(c, fbp)

        # Software pipeline: the two head chunks' mm1s interleave by
        # fb-pair so each arriving w1 slice feeds two PE groups (PE
        # covers the w1 DMA stream with no idle); afterwards mm1 stays
        # two chunks ahead of mm2 so the w2/w1B streams land in time.
        if NCT > 1:
            alloc_h(0)
            alloc_h(1)
            for fbp in range(FB // 2):
                emit_mm1_group(0, fbp)
                emit_mm1_group(1, fbp)
        else:
            emit_mm1(0)
        for c in range(NCT):
            emit_mm2(c)
            if c + 2 < NCT:
                emit_mm1(c + 2)
    return nc


def _get_nc(S0, S1, L0, L1, fuse1=True, fuse2=True):
    key = (S0, S1, L0, L1, fuse1, fuse2)
    if key not in _CACHE:
        nc = _build_nc(S0, S1, L0, L1, fuse1, fuse2)
        nc.finalize()
        _CACHE[key] = nc
    return _CACHE[key]


def _split8(a):
    hi = a.astype(fp8)
    lo = (a - hi.astype(np.float32)).astype(fp8)
    return hi, lo


def _x_layout(x8, chunks, idxA, idxB, S0):
    """[H, T] fp8 + chunk list -> [P, NCT*2048] in [p, c, j, i, t] layout,
    one 256-padded block per chunk in chunk-list order."""
    cols = np.zeros(len(chunks) * CK, dtype=np.int64)
    for ci, (off, csz, seg) in enumerate(chunks):
        idx = idxA if seg == 0 else idxB
        pos = off - (0 if seg == 0 else S0)
        take = idx[pos:min(pos + csz, len(idx))]
        cols[ci * CK:ci * CK + len(take)] = take
    g = x8[:, cols]                                  # [H, NCT*256]
    NCT_ = len(chunks)
    g = g.reshape(J, 2, P, NCT_, CK)                 # [j, i, p, c, t]
    return np.ascontiguousarray(
        g.transpose(2, 3, 0, 1, 4).reshape(P, NCT_ * CK * 8))


def _w1_layout(a):
    """[H, FH] -> [P, FB*8*P] as [p, fb, j, i, f]."""
    return np.ascontiguousarray(
        a.reshape(J, 2, P, FB, P).transpose(2, 3, 0, 1, 4).reshape(P, -1))


def _w2_layout(a):
    """[FH, H] -> [P, (H//128)*G*2*128] as [p, hb, g, i, h]."""
    return np.ascontiguousarray(
        a.reshape(G, 2, P, H // P, P).transpose(2, 3, 0, 1, 4).reshape(P, -1))


def dispatch(hidden_states, router_w, router_b):
    """Host router: exact fp32 softmax top-2 + renormalized weights."""
    x = np.asarray(hidden_states, dtype=np.float32).reshape(T, H)
    logits = x @ np.asarray(router_w, dtype=np.float32)
    logits = logits + np.asarray(router_b, dtype=np.float32)
    part = np.argpartition(logits, E - 2, axis=1)[:, E - 2:]     # top-2 ids
    lg = np.take_along_axis(logits, part, axis=1)                # [T, 2]
    m = lg.max(axis=1, keepdims=True)
    e = np.exp(lg - m)
    wslot = e / e.sum(axis=1, keepdims=True)                     # [T, 2]
    idx_lists, wts = [], []
    for m_ in range(E):
        hit = part == m_
        rows = np.where(hit.any(axis=1))[0]
        idx_lists.append(rows)
        wts.append((wslot * hit)[rows].sum(axis=1))
    return x, idx_lists, wts


def _pad128(n):
    return max(P, ((n + P - 1) // P) * P)


def make_in_maps(hidden_states, router_w, router_b, w1, b1, w2, b2):
    x, idx_lists, wts = dispatch(hidden_states, router_w, router_b)
    loads = np.array([len(ix) for ix in idx_lists])
    order = np.argsort(-loads, kind="stable")
    hots, colds = order[:4], order[4:]
    L0 = max(int(loads[hots].max()), 1)
    L1 = max(int(loads[colds].max()), 1)
    S0 = _pad128(L0)
    S1 = _pad128(L1)
    C = S0 + S1
    xt = np.ascontiguousarray(x.T)                   # [H, T] f32
    x8h, x8l = _split8(xt)
    w1 = np.asarray(w1, dtype=np.float32)
    w2 = np.asarray(w2, dtype=np.float32)
    b1 = np.asarray(b1, dtype=np.float32)
    b2 = np.asarray(b2, dtype=np.float32)
    fuse1 = not b1.any()
    fuse2 = not b2.any()
    pairs = list(zip(hots, colds))
    in_maps = []
    for eA, eB in pairs:
        ixA, ixB = idx_lists[eA], idx_lists[eB]
        chunks = _chunks(S0, S1, L0, L1)
        xh_full = _x_layout(x8h, chunks, ixA, ixB, S0)
        xl_full = _x_layout(x8l, chunks, ixA, ixB, S0)
        wcol = np.zeros(C, dtype=np.float32)
        wcol[:len(ixA)] = wts[eA] / 256.0
        wcol[S0:S0 + len(ixB)] = wts[eB] / 256.0
        # combine weights ride the free (token) dim: replicate across rows
        wdv_m = np.ascontiguousarray(np.broadcast_to(wcol, (P, C)))
        for side in range(2):
            fsl = slice(side * FH, (side + 1) * FH)
            im = {"xh": xh_full, "xl": xl_full, "wdv": wdv_m}
            for s, e_ in (("a", eA), ("b", eB)):
                hi1, lo1 = _split8(w1[e_][:, fsl] * 256.0)
                im[f"w1{s}h"], im[f"w1{s}l"] = _w1_layout(hi1), _w1_layout(lo1)
                hi2, lo2 = _split8(w2[e_][fsl, :] * 256.0)
                im[f"w2{s}h"], im[f"w2{s}l"] = _w2_layout(hi2), _w2_layout(lo2)
            b1m = np.stack([
                b1[eA][fsl].reshape(FB, P).T, b1[eB][fsl].reshape(FB, P).T])
            im["b1d"] = np.ascontiguousarray(
                b1m.transpose(1, 0, 2).reshape(P, 2 * FB))
            # b2 is added once per token: by side 0 only. [p, seg, hb]
            if side == 0:
                b2m = np.stack([
                    (b2[eA] * 256.0).reshape(H // P, P).T,
                    (b2[eB] * 256.0).reshape(H // P, P).T])
            else:
                b2m = np.zeros((2, P, H // P), dtype=np.float32)
            im["b2c"] = np.ascontiguousarray(
                np.asarray(b2m, dtype=np.float32).transpose(1, 0, 2)
                .reshape(P, 2 * (H // P)))
            in_maps.append(im)
    return in_maps, idx_lists, (S0, S1, L0, L1), pairs, fuse1, fuse2


def run_device(in_maps, caps, fuse1=True, fuse2=True):
    from concourse.bass_utils import run_bass_kernel_spmd

    nc = _get_nc(*caps, fuse1, fuse2)
    res = run_bass_kernel_spmd(nc, in_maps, core_ids=list(range(E)))
    return res.results


def kernel(hidden_states, router_w, router_b, w1, b1, w2, b2):
    in_maps, idx_lists, caps, pairs, fuse1, fuse2 = make_in_maps(
        hidden_states, router_w, router_b, w1, b1, w2, b2)
    S0 = caps[0]
    # One retry guards against a rare transient execution glitch observed on
    # the very first load of a freshly compiled NEFF (garbage ~1e35 values);
    # a healthy output has absmax of a few units.
    last_err = None
    acc = None
    for attempt in range(3):
        try:
            results = run_device(in_maps, caps, fuse1, fuse2)
        except Exception as e:  # transient NRT/axon failures observed
            last_err = e
            import time as _time
            _time.sleep(10)
            continue
        acc = np.zeros((T, H), dtype=np.float32)
        for i, (eA, eB) in enumerate(pairs):
            y0 = np.asarray(results[2 * i]["yc"], dtype=np.float32)
            y1 = np.asarray(results[2 * i + 1]["yc"], dtype=np.float32)
            ysum = (y0 + y1).T                       # [H, C] -> [C, H]
            ixA, ixB = idx_lists[eA], idx_lists[eB]
            acc[ixA] += ysum[:len(ixA)]
            acc[ixB] += ysum[S0:S0 + len(ixB)]
        if np.isfinite(acc).all() and np.abs(acc).max() < 1e4:
            return acc.reshape(B, S, H)
    if acc is None and last_err is not None:
        raise last_err
    return acc.reshape(B, S, H)


# revision 50
# speedup vs baseline: 1.0024x; 1.0024x over previous
"""MoE layer (8 experts, top-2) on 8 TRN2 NeuronCores.

Strategy (expert-parallel with pairwise tensor-split, fp8 DoubleRow FFN):
  - Host computes the router exactly (fp32 numpy), does the top-2
    dispatch and ships the per-token combine weight, so the device does
    only the expert FFN.
  - Experts are sorted by load and split hot/cold; pair i = (hot_i,
    cold_i) is served by cores (2i, 2i+1), each holding one F-half of
    BOTH experts' weights. Both cores process the pair's full token
    list (segment A = hot tokens padded to S0, segment B = cold tokens
    padded to S1, S0/S1 shared across pairs so the SPMD program is
    uniform); the host adds the two half-F partial outputs. This costs
    (S0+S1)/2 full-F token-equivalents per core instead of S0 — load
    balancing that cuts PE time ~6%.
  - FFN runs on the PE in fp8-e4m3 DoubleRow mode (two 128-row k-tiles
    per instruction) with full error compensation: every operand is
    split into hi + lo fp8 parts (lo = residual of the hi quantization)
    and each matmul accumulates three passes in one PSUM group:
        hi@hi + lo@hi + hi@lo    (the lo@lo term is negligible)
    Weight tensors are pre-scaled by 256 on the host so every pass
    lands at the same power-of-2 scale; the 1/256 is folded into the
    gelu scale (mm1) and the combine weight (mm2).
  - Output f-blocks are processed in pairs sharing one [128, 2, 256]
    PSUM bank so ACT/DVE/DMA instruction counts stay half of PE's.
  - h = gelu(x @ w1 + b1) is written twice by the scalar engine (fp8 hi
    + f32), the DVE derives the fp8 lo residual.
  - The two head chunks' mm1s interleave by f-block so the PE covers
    the w1 DMA stream with no idle.
"""

from contextlib import ExitStack

import ml_dtypes
import numpy as np

P = 128
B, S, H, F, E = 2, 2048, 1024, 4096, 8
T = B * S            # 4096 tokens
FH = F // 2          # 2048 per-core F half
J = H // 256         # 4  mm1 k-tile pairs
G = FH // 256        # 8  mm2 k-tile pairs
FB = FH // P         # 16 mm1 output f-blocks
HB = H // 256        # 4  mm2 output h-blocks
CK = 256             # token chunk

fp8 = ml_dtypes.float8_e4m3fn

_CACHE = {}


def _chunks(S0, S1, L0, L1):
    """[(offset_in_C, csz, seg)]: 256-token chunks per segment, with the
    last chunk trimmed to the segment's actual max load (L) — matmul
    cost is proportional to the moving width, so tokens between L and
    the 128-padded capacity S are never computed. Partial chunks go
    last (smallest at the very end) so the end-of-program output drain
    trails the narrowest possible tile."""
    full, partial = [], []
    for seg, (base, load) in enumerate([(0, L0), (S0, L1)]):
        t0 = 0
        while t0 < load:
            csz = min(CK, load - t0)
            (full if csz == CK else partial).append((base + t0, csz, seg))
            t0 += csz
    partial.sort(key=lambda t: -t[1])
    return full + partial


def _build_nc(S0, S1, L0, L1, fuse1, fuse2):
    import concourse.mybir as mybir
    import concourse.tile as tile
    from concourse import bacc

    dt = mybir.dt
    AF = mybir.ActivationFunctionType
    ALU = mybir.AluOpType
    PM = mybir.MatmulPerfMode

    C = S0 + S1
    chunks = _chunks(S0, S1, L0, L1)
    NCT = len(chunks)
    TTS = C // P                     # token tiles

    nc = bacc.Bacc(
        "TRN2", target_bir_lowering=False, debug=False, num_devices=E)

    xh = nc.declare_dram_parameter("xh", [P, NCT * 2048], dt.float8e4, isOutput=False)
    xl = nc.declare_dram_parameter("xl", [P, NCT * 2048], dt.float8e4, isOutput=False)
    w1p = {}
    w2p = {}
    HBK = H // P                     # 8 mm2 output h-blocks of 128
    for s in "ab":
        w1p[s] = [nc.declare_dram_parameter(f"w1{s}{t}", [P, FB * 8 * P],
                                            dt.float8e4, isOutput=False)
                  for t in "hl"]
        w2p[s] = [nc.declare_dram_parameter(f"w2{s}{t}", [P, HBK * G * 2 * P],
                                            dt.float8e4, isOutput=False)
                  for t in "hl"]
    b1d = nc.declare_dram_parameter("b1d", [P, 2 * FB], dt.float32, isOutput=False)
    b2c = nc.declare_dram_parameter("b2c", [P, 2 * HBK], dt.float32, isOutput=False)
    wdv = nc.declare_dram_parameter("wdv", [P, C], dt.float32, isOutput=False)
    yc = nc.declare_dram_parameter("yc", [H, C], dt.float32, isOutput=True)

    xh_r = xh.rearrange("p (c j i t) -> p c j i t", c=NCT, j=J, i=2)
    xl_r = xl.rearrange("p (c j i t) -> p c j i t", c=NCT, j=J, i=2)
    w1r = {s: [a.rearrange("p (fb j i f) -> p fb j i f", fb=FB, j=J, i=2)
               for a in w1p[s]] for s in "ab"}
    w2r = {s: [a.rearrange("p (hb g i h) -> p hb g i h", hb=HBK, g=G, i=2)
               for a in w2p[s]] for s in "ab"}
    yc_r = yc.rearrange("(b p) t -> p b t", p=P)

    with ExitStack() as ctx:
        tc = ctx.enter_context(tile.TileContext(nc))
        const = ctx.enter_context(tc.tile_pool(name="const", bufs=1))
        # All DMAs issue on the single SP queue and a waiting DMA holds
        # the SP sequencer, so pools backing DMA-adjacent tiles must be
        # deep enough that no DMA ever waits on buffer reuse: x tiles
        # that do recycle buffers are loaded at the END of the input
        # stream, and the ob pool is deep enough that mm2 output muls
        # never wait for an output DMA to drain.
        xpool = ctx.enter_context(tc.tile_pool(name="xt", bufs=min(2 * NCT, 12)))
        h8pool = ctx.enter_context(tc.tile_pool(name="h8", bufs=2))
        hlpool = ctx.enter_context(tc.tile_pool(name="hl", bufs=2))
        gpool = ctx.enter_context(tc.tile_pool(name="g32", bufs=4))
        p1pool = ctx.enter_context(tc.tile_pool(name="p1", bufs=4, space="PSUM"))
        p2pool = ctx.enter_context(tc.tile_pool(name="p2", bufs=4, space="PSUM"))
        opool = ctx.enter_context(tc.tile_pool(name="ob", bufs=8))

        # ---- DMA schedule: head-chunk x first, then w1A in fine slices
        # (hi/lo interleaved), w2A, w1B, w2B, with remaining x chunks
        # threaded between. ----
        xh_s = [None] * NCT
        xl_s = [None] * NCT

        def load_x(c):
            xh_s[c] = xpool.tile([P, J, 2, CK], dt.float8e4, name="xt")
            xl_s[c] = xpool.tile([P, J, 2, CK], dt.float8e4, name="xt")
            nc.sync.dma_start(xh_s[c][:], xh_r[:, c])
            nc.sync.dma_start(xl_s[c][:], xl_r[:, c])

        b1_s = const.tile([P, 2, FB], dt.float32)
        wdv_s = const.tile([P, C], dt.float32)
        w1_s = {}
        w2_s = {}
        for s in "ab":
            w1_s[s] = [const.tile([P, FB, J, 2, P], dt.float8e4, name=f"w1{s}{t}")
                       for t in "hl"]
            w2_s[s] = [const.tile([P, HBK, G, 2, P], dt.float8e4, name=f"w2{s}{t}")
                       for t in "hl"]

        # PE p-state warmup: dummy DoubleRow matmuls on a zeroed tile
        # burn the cost model's clock ramp (~3us of accumulated busy
        # before full speed) during the otherwise-idle head DMA wait.
        wut = const.tile([P, 2, 256], dt.float8e4)
        nc.vector.memset(wut[:], 0)
        for i in range(32):
            pw = p1pool.tile([P, 2, CK], dt.float32, name="p1")
            nc.tensor.matmul(
                pw[:, 0], wut[:, :, :P], wut[:], start=True, stop=True,
                perf_mode=PM.DoubleRow)

        load_x(0)
        for si, (fb0, nfb) in enumerate([(0, 2), (2, 2), (4, 4), (8, 4), (12, 4)]):
            sl = slice(fb0, fb0 + nfb)
            nc.sync.dma_start(w1_s["a"][0][:, sl], w1r["a"][0][:, sl])
            nc.sync.dma_start(w1_s["a"][1][:, sl], w1r["a"][1][:, sl])
            if si == 0:
                nc.sync.dma_start(b1_s[:], b1d.rearrange("p (s f) -> p s f", s=2))
                if NCT > 1:
                    load_x(1)
        nc.sync.dma_start(wdv_s[:], wdv[:])
        b2c_s = None
        if not fuse2:
            b2c_s = const.tile([P, 2, HBK], dt.float32)
        # x chunks that get fresh buffers interleave with the weight
        # stream; the tail chunks (recycled buffers, whose DMA waits for
        # the earlier reader) go last so the wait blocks nothing.
        nfresh = min(2 * NCT, 12) // 2
        nxt = 2
        for hb in range(0, HBK, 2):
            sl = slice(hb, hb + 2)
            nc.sync.dma_start(w2_s["a"][0][:, sl], w2r["a"][0][:, sl])
            nc.sync.dma_start(w2_s["a"][1][:, sl], w2r["a"][1][:, sl])
            if hb == 0 and not fuse2:
                nc.sync.dma_start(b2c_s[:], b2c.rearrange("p (s h) -> p s h", s=2))
            if nxt < nfresh:
                load_x(nxt)
                nxt += 1
        for fb0 in range(0, FB, 4):
            sl = slice(fb0, fb0 + 4)
            nc.sync.dma_start(w1_s["b"][0][:, sl], w1r["b"][0][:, sl])
            nc.sync.dma_start(w1_s["b"][1][:, sl], w1r["b"][1][:, sl])
            if nxt < nfresh:
                load_x(nxt)
                nxt += 1
        for hb in range(0, HBK, 2):
            sl = slice(hb, hb + 2)
            nc.sync.dma_start(w2_s["b"][0][:, sl], w2r["b"][0][:, sl])
            nc.sync.dma_start(w2_s["b"][1][:, sl], w2r["b"][1][:, sl])
            if nxt < nfresh:
                load_x(nxt)
                nxt += 1
        while nxt < NCT:
            load_x(nxt)
            nxt += 1

        hs = [None] * NCT

        def alloc_h(c):
            h8 = h8pool.tile([P, G, 2, CK], dt.float8e4, name="h8")
            hl = hlpool.tile([P, G, 2, CK], dt.float8e4, name="hl")
            hs[c] = (h8, hl)

        def emit_mm1_group(c, fbp):
            off, csz, seg = chunks[c]
            sk = "ab"[seg]
            w1hs, w1ls = w1_s[sk]
            xht, xlt = xh_s[c], xl_s[c]
            h8, hl = hs[c]
            ps = p1pool.tile([P, 2, CK], dt.float32, name="p1")
            for half in range(2):
                fb = 2 * fbp + half
                reg = ps[:, half, :csz]
                for j in range(J):
                    nc.tensor.matmul(
                        reg, w1hs[:, fb, j], xht[:, j, :, :csz],
                        start=(j == 0), stop=False, perf_mode=PM.DoubleRow)
                for j in range(J):
                    nc.tensor.matmul(
                        reg, w1hs[:, fb, j], xlt[:, j, :, :csz],
                        start=False, stop=False, perf_mode=PM.DoubleRow)
                for j in range(J):
                    nc.tensor.matmul(
                        reg, w1ls[:, fb, j], xht[:, j, :, :csz],
                        start=False, stop=(j == J - 1), perf_mode=PM.DoubleRow)
            g32 = gpool.tile([P, 2, CK], dt.float32, name="g32")
            h8v = h8[:, fbp, :, :csz]
            if fuse1:
                nc.scalar.activation(
                    g32[:, :, :csz], ps[:, :, :csz], AF.Gelu,
                    bias=0.0, scale=1.0 / 256)
                nc.scalar.activation(
                    h8v, ps[:, :, :csz], AF.Gelu, bias=0.0, scale=1.0 / 256)
            else:
                for half in range(2):
                    fb = 2 * fbp + half
                    nc.scalar.activation(
                        g32[:, half, :csz], ps[:, half, :csz], AF.Gelu,
                        bias=b1_s[:, seg, fb:fb + 1], scale=1.0 / 256)
                    nc.scalar.activation(
                        h8[:, fbp, half, :csz], ps[:, half, :csz], AF.Gelu,
                        bias=b1_s[:, seg, fb:fb + 1], scale=1.0 / 256)
            nc.vector.tensor_tensor(
                hl[:, fbp, :, :csz], g32[:, :, :csz], h8v, ALU.subtract)

        def emit_mm2(c):
            # Tokens ride the FREE dim (stationary w2, moving h), so mm2
            # cost is proportional to the chunk's actual token count and
            # the per-token combine weight is a plain elementwise mult.
            off, csz, seg = chunks[c]
            sk = "ab"[seg]
            w2hs, w2ls = w2_s[sk]
            h8, hl = hs[c]
            for hbp in range(HBK // 2):
                ps2 = p2pool.tile([P, 2, CK], dt.float32, name="p2")
                for half in range(2):
                    hb = 2 * hbp + half
                    reg = ps2[:, half, :csz]
                    for g in range(G):
                        nc.tensor.matmul(
                            reg, w2hs[:, hb, g], h8[:, g, :, :csz],
                            start=(g == 0), stop=False, perf_mode=PM.DoubleRow)
                    for g in range(G):
                        nc.tensor.matmul(
                            reg, w2hs[:, hb, g], hl[:, g, :, :csz],
                            start=False, stop=False, perf_mode=PM.DoubleRow)
                    for g in range(G):
                        nc.tensor.matmul(
                            reg, w2ls[:, hb, g], h8[:, g, :, :csz],
                            start=False, stop=(g == G - 1), perf_mode=PM.DoubleRow)
                ob = opool.tile([P, 2, CK], dt.float32, name="ob")
                for half in range(2):
                    hb = 2 * hbp + half
                    if fuse2:
                        nc.vector.tensor_tensor(
                            ob[:, half, :csz], ps2[:, half, :csz],
                            wdv_s[:, off:off + csz], ALU.mult)
                    else:
                        nc.vector.tensor_scalar_add(
                            ob[:, half, :csz], ps2[:, half, :csz],
                            b2c_s[:, seg, hb:hb + 1])
                        nc.vector.tensor_tensor(
                            ob[:, half, :csz], ob[:, half, :csz],
                            wdv_s[:, off:off + csz], ALU.mult)
                nc.sync.dma_start(
                    yc_r[:, 2 * hbp:2 * hbp + 2, off:off + csz],
                    ob[:, :, :csz])

        def emit_mm1(c):
            alloc_h(c)
            for fbp in range(FB // 2):
                emit_mm1_group(c, fbp)

        # Software pipeline: the two head chunks' mm1s interleave by
        # fb-pair so each arriving w1 slice feeds two PE groups (PE
        # covers the w1 DMA stream with no idle); afterwards mm1 stays
        # two chunks ahead of mm2 so the w2/w1B streams land in time.
        if NCT > 1:
            alloc_h(0)
            alloc_h(1)
            for fbp in range(FB // 2):
                emit_mm1_group(0, fbp)
                emit_mm1_group(1, fbp)
        else:
            emit_mm1(0)
        for c in range(NCT):
            emit_mm2(c)
            if c + 2 < NCT:
                emit_mm1(c + 2)
    return nc


def _get_nc(S0, S1, L0, L1, fuse1=True, fuse2=True):
    key = (S0, S1, L0, L1, fuse1, fuse2)
    if key not in _CACHE:
        nc = _build_nc(S0, S1, L0, L1, fuse1, fuse2)
        nc.finalize()
        _CACHE[key] = nc
    return _CACHE[key]


def _split8(a):
    hi = a.astype(fp8)
    lo = (a - hi.astype(np.float32)).astype(fp8)
    return hi, lo


def _x_layout(x8, chunks, idxA, idxB, S0):
    """[H, T] fp8 + chunk list -> [P, NCT*2048] in [p, c, j, i, t] layout,
    one 256-padded block per chunk in chunk-list order."""
    cols = np.zeros(len(chunks) * CK, dtype=np.int64)
    for ci, (off, csz, seg) in enumerate(chunks):
        idx = idxA if seg == 0 else idxB
        pos = off - (0 if seg == 0 else S0)
        take = idx[pos:min(pos + csz, len(idx))]
        cols[ci * CK:ci * CK + len(take)] = take
    g = x8[:, cols]                                  # [H, NCT*256]
    NCT_ = len(chunks)
    g = g.reshape(J, 2, P, NCT_, CK)                 # [j, i, p, c, t]
    return np.ascontiguousarray(
        g.transpose(2, 3, 0, 1, 4).reshape(P, NCT_ * CK * 8))


def _w1_layout(a):
    """[H, FH] -> [P, FB*8*P] as [p, fb, j, i, f]."""
    return np.ascontiguousarray(
        a.reshape(J, 2, P, FB, P).transpose(2, 3, 0, 1, 4).reshape(P, -1))


def _w2_layout(a):
    """[FH, H] -> [P, (H//128)*G*2*128] as [p, hb, g, i, h]."""
    return np.ascontiguousarray(
        a.reshape(G, 2, P, H // P, P).transpose(2, 3, 0, 1, 4).reshape(P, -1))


def dispatch(hidden_states, router_w, router_b):
    """Host router: exact fp32 softmax top-2 + renormalized weights."""
    x = np.asarray(hidden_states, dtype=np.float32).reshape(T, H)
    logits = x @ np.asarray(router_w, dtype=np.float32)
    logits = logits + np.asarray(router_b, dtype=np.float32)
    part = np.argpartition(logits, E - 2, axis=1)[:, E - 2:]     # top-2 ids
    lg = np.take_along_axis(logits, part, axis=1)                # [T, 2]
    m = lg.max(axis=1, keepdims=True)
    e = np.exp(lg - m)
    wslot = e / e.sum(axis=1, keepdims=True)                     # [T, 2]
    idx_lists, wts = [], []
    for m_ in range(E):
        hit = part == m_
        rows = np.where(hit.any(axis=1))[0]
        idx_lists.append(rows)
        wts.append((wslot * hit)[rows].sum(axis=1))
    return x, idx_lists, wts


def _pad128(n):
    return max(P, ((n + P - 1) // P) * P)


def make_in_maps(hidden_states, router_w, router_b, w1, b1, w2, b2):
    x, idx_lists, wts = dispatch(hidden_states, router_w, router_b)
    loads = np.array([len(ix) for ix in idx_lists])
    order = np.argsort(-loads, kind="stable")
    hots, colds = order[:4], order[4:]
    L0 = max(int(loads[hots].max()), 1)
    L1 = max(int(loads[colds].max()), 1)
    S0 = _pad128(L0)
    S1 = _pad128(L1)
    C = S0 + S1
    xt = np.ascontiguousarray(x.T)                   # [H, T] f32
    x8h, x8l = _split8(xt)
    w1 = np.asarray(w1, dtype=np.float32)
    w2 = np.asarray(w2, dtype=np.float32)
    b1 = np.asarray(b1, dtype=np.float32)
    b2 = np.asarray(b2, dtype=np.float32)
    fuse1 = not b1.any()
    fuse2 = not b2.any()
    pairs = list(zip(hots, colds))
    in_maps = []
    for eA, eB in pairs:
        ixA, ixB = idx_lists[eA], idx_lists[eB]
        chunks = _chunks(S0, S1, L0, L1)
        xh_full = _x_layout(x8h, chunks, ixA, ixB, S0)
        xl_full = _x_layout(x8l, chunks, ixA, ixB, S0)
        wcol = np.zeros(C, dtype=np.float32)
        wcol[:len(ixA)] = wts[eA] / 256.0
        wcol[S0:S0 + len(ixB)] = wts[eB] / 256.0
        # combine weights ride the free (token) dim: replicate across rows
        wdv_m = np.ascontiguousarray(np.broadcast_to(wcol, (P, C)))
        for side in range(2):
            fsl = slice(side * FH, (side + 1) * FH)
            im = {"xh": xh_full, "xl": xl_full, "wdv": wdv_m}
            for s, e_ in (("a", eA), ("b", eB)):
                hi1, lo1 = _split8(w1[e_][:, fsl] * 256.0)
                im[f"w1{s}h"], im[f"w1{s}l"] = _w1_layout(hi1), _w1_layout(lo1)
                hi2, lo2 = _split8(w2[e_][fsl, :] * 256.0)
                im[f"w2{s}h"], im[f"w2{s}l"] = _w2_layout(hi2), _w2_layout(lo2)
            b1m = np.stack([
                b1[eA][fsl].reshape(FB, P).T, b1[eB][fsl].reshape(FB, P).T])
            im["b1d"] = np.ascontiguousarray(
                b1m.transpose(1, 0, 2).reshape(P, 2 * FB))
            # b2 is added once per token: by side 0 only. [p, seg, hb]
            if side == 0:
                b2m = np.stack([
                    (b2[eA] * 256.0).reshape(H // P, P).T,
                    (b2[eB] * 256.0).reshape(H // P, P).T])
            else:
                b2m = np.zeros((2, P, H // P), dtype=np.float32)
            im["b2c"] = np.ascontiguousarray(
                np.asarray(b2m, dtype=np.float32).transpose(1, 0, 2)
                .reshape(P, 2 * (H // P)))
            in_maps.append(im)
    return in_maps, idx_lists, (S0, S1, L0, L1), pairs, fuse1, fuse2


def run_device(in_maps, caps, fuse1=True, fuse2=True):
    from concourse.bass_utils import run_bass_kernel_spmd

    nc = _get_nc(*caps, fuse1, fuse2)
    res = run_bass_kernel_spmd(nc, in_maps, core_ids=list(range(E)))
    return res.results


def kernel(hidden_states, router_w, router_b, w1, b1, w2, b2):
    in_maps, idx_lists, caps, pairs, fuse1, fuse2 = make_in_maps(
        hidden_states, router_w, router_b, w1, b1, w2, b2)
    S0 = caps[0]
    # One retry guards against a rare transient execution glitch observed on
    # the very first load of a freshly compiled NEFF (garbage ~1e35 values);
    # a healthy output has absmax of a few units.
    last_err = None
    acc = None
    for attempt in range(3):
        try:
            results = run_device(in_maps, caps, fuse1, fuse2)
        except Exception as e:  # transient NRT/axon failures observed
            last_err = e
            import time as _time
            _time.sleep(10)
            continue
        acc = np.zeros((T, H), dtype=np.float32)
        for i, (eA, eB) in enumerate(pairs):
            y0 = np.asarray(results[2 * i]["yc"], dtype=np.float32)
            y1 = np.asarray(results[2 * i + 1]["yc"], dtype=np.float32)
            ysum = (y0 + y1).T                       # [H, C] -> [C, H]
            ixA, ixB = idx_lists[eA], idx_lists[eB]
            acc[ixA] += ysum[:len(ixA)]
            acc[ixB] += ysum[S0:S0 + len(ixB)]
        if np.isfinite(acc).all() and np.abs(acc).max() < 1e4:
            return acc.reshape(B, S, H)
    if acc is None and last_err is not None:
        raise last_err
    return acc.reshape(B, S, H)


# revision 51
# speedup vs baseline: 1.0035x; 1.0010x over previous
"""MoE layer (8 experts, top-2) on 8 TRN2 NeuronCores.

Strategy (expert-parallel with pairwise tensor-split, fp8 DoubleRow FFN):
  - Host computes the router exactly (fp32 numpy), does the top-2
    dispatch and ships the per-token combine weight, so the device does
    only the expert FFN.
  - Experts are sorted by load and split hot/cold; pair i = (hot_i,
    cold_i) is served by cores (2i, 2i+1), each holding one F-half of
    BOTH experts' weights. Both cores process the pair's full token
    list (segment A = hot tokens padded to S0, segment B = cold tokens
    padded to S1, S0/S1 shared across pairs so the SPMD program is
    uniform); the host adds the two half-F partial outputs. This costs
    (S0+S1)/2 full-F token-equivalents per core instead of S0 — load
    balancing that cuts PE time ~6%.
  - FFN runs on the PE in fp8-e4m3 DoubleRow mode (two 128-row k-tiles
    per instruction) with full error compensation: every operand is
    split into hi + lo fp8 parts (lo = residual of the hi quantization)
    and each matmul accumulates three passes in one PSUM group:
        hi@hi + lo@hi + hi@lo    (the lo@lo term is negligible)
    Weight tensors are pre-scaled by 256 on the host so every pass
    lands at the same power-of-2 scale; the 1/256 is folded into the
    gelu scale (mm1) and the combine weight (mm2).
  - Output f-blocks are processed in pairs sharing one [128, 2, 256]
    PSUM bank so ACT/DVE/DMA instruction counts stay half of PE's.
  - h = gelu(x @ w1 + b1) is written twice by the scalar engine (fp8 hi
    + f32), the DVE derives the fp8 lo residual.
  - The two head chunks' mm1s interleave by f-block so the PE covers
    the w1 DMA stream with no idle.
"""

from contextlib import ExitStack

import ml_dtypes
import numpy as np

P = 128
B, S, H, F, E = 2, 2048, 1024, 4096, 8
T = B * S            # 4096 tokens
FH = F // 2          # 2048 per-core F half
J = H // 256         # 4  mm1 k-tile pairs
G = FH // 256        # 8  mm2 k-tile pairs
FB = FH // P         # 16 mm1 output f-blocks
HB = H // 256        # 4  mm2 output h-blocks
CK = 256             # token chunk

fp8 = ml_dtypes.float8_e4m3fn

_CACHE = {}


def _chunks(S0, S1, L0, L1):
    """[(offset_in_C, csz, seg)]: 256-token chunks per segment, with the
    last chunk trimmed to the segment's actual max load (L) — matmul
    cost is proportional to the moving width, so tokens between L and
    the 128-padded capacity S are never computed. Partial chunks go
    last (smallest at the very end) so the end-of-program output drain
    trails the narrowest possible tile."""
    full, partial = [], []
    for seg, (base, load) in enumerate([(0, L0), (S0, L1)]):
        t0 = 0
        while t0 < load:
            csz = min(CK, load - t0)
            (full if csz == CK else partial).append((base + t0, csz, seg))
            t0 += csz
    partial.sort(key=lambda t: -t[1])
    return full + partial


def _build_nc(S0, S1, L0, L1, fuse1, fuse2):
    import concourse.mybir as mybir
    import concourse.tile as tile
    from concourse import bacc

    dt = mybir.dt
    AF = mybir.ActivationFunctionType
    ALU = mybir.AluOpType
    PM = mybir.MatmulPerfMode

    C = S0 + S1
    chunks = _chunks(S0, S1, L0, L1)
    NCT = len(chunks)
    TTS = C // P                     # token tiles

    nc = bacc.Bacc(
        "TRN2", target_bir_lowering=False, debug=False, num_devices=E)

    xh = nc.declare_dram_parameter("xh", [P, NCT * 2048], dt.float8e4, isOutput=False)
    xl = nc.declare_dram_parameter("xl", [P, NCT * 2048], dt.float8e4, isOutput=False)
    w1p = {}
    w2p = {}
    HBK = H // P                     # 8 mm2 output h-blocks of 128
    for s in "ab":
        w1p[s] = [nc.declare_dram_parameter(f"w1{s}{t}", [P, FB * 8 * P],
                                            dt.float8e4, isOutput=False)
                  for t in "hl"]
        w2p[s] = [nc.declare_dram_parameter(f"w2{s}{t}", [P, HBK * G * 2 * P],
                                            dt.float8e4, isOutput=False)
                  for t in "hl"]
    b1d = nc.declare_dram_parameter("b1d", [P, 2 * FB], dt.float32, isOutput=False)
    b2c = nc.declare_dram_parameter("b2c", [P, 2 * HBK], dt.float32, isOutput=False)
    wdv = nc.declare_dram_parameter("wdv", [P, C], dt.float32, isOutput=False)
    yc = nc.declare_dram_parameter("yc", [H, C], dt.float32, isOutput=True)

    xh_r = xh.rearrange("p (c j i t) -> p c j i t", c=NCT, j=J, i=2)
    xl_r = xl.rearrange("p (c j i t) -> p c j i t", c=NCT, j=J, i=2)
    w1r = {s: [a.rearrange("p (fb j i f) -> p fb j i f", fb=FB, j=J, i=2)
               for a in w1p[s]] for s in "ab"}
    w2r = {s: [a.rearrange("p (hb g i h) -> p hb g i h", hb=HBK, g=G, i=2)
               for a in w2p[s]] for s in "ab"}
    yc_r = yc.rearrange("(b p) t -> p b t", p=P)

    with ExitStack() as ctx:
        tc = ctx.enter_context(tile.TileContext(nc))
        const = ctx.enter_context(tc.tile_pool(name="const", bufs=1))
        # All DMAs issue on the single SP queue and a waiting DMA holds
        # the SP sequencer, so pools backing DMA-adjacent tiles must be
        # deep enough that no DMA ever waits on buffer reuse: x tiles
        # that do recycle buffers are loaded at the END of the input
        # stream, and the ob pool is deep enough that mm2 output muls
        # never wait for an output DMA to drain.
        xpool = ctx.enter_context(tc.tile_pool(name="xt", bufs=min(2 * NCT, 12)))
        h8pool = ctx.enter_context(tc.tile_pool(name="h8", bufs=2))
        hlpool = ctx.enter_context(tc.tile_pool(name="hl", bufs=2))
        gpool = ctx.enter_context(tc.tile_pool(name="g32", bufs=4))
        p1pool = ctx.enter_context(tc.tile_pool(name="p1", bufs=4, space="PSUM"))
        p2pool = ctx.enter_context(tc.tile_pool(name="p2", bufs=4, space="PSUM"))
        opool = ctx.enter_context(tc.tile_pool(name="ob", bufs=8))

        # ---- DMA schedule: head-chunk x first, then w1A in fine slices
        # (hi/lo interleaved), w2A, w1B, w2B, with remaining x chunks
        # threaded between. ----
        xh_s = [None] * NCT
        xl_s = [None] * NCT

        def load_x(c):
            xh_s[c] = xpool.tile([P, J, 2, CK], dt.float8e4, name="xt")
            xl_s[c] = xpool.tile([P, J, 2, CK], dt.float8e4, name="xt")
            nc.sync.dma_start(xh_s[c][:], xh_r[:, c])
            nc.sync.dma_start(xl_s[c][:], xl_r[:, c])

        b1_s = const.tile([P, 2, FB], dt.float32)
        wdv_s = const.tile([P, C], dt.float32)
        w1_s = {}
        w2_s = {}
        for s in "ab":
            w1_s[s] = [const.tile([P, FB, J, 2, P], dt.float8e4, name=f"w1{s}{t}")
                       for t in "hl"]
            w2_s[s] = [const.tile([P, HBK, G, 2, P], dt.float8e4, name=f"w2{s}{t}")
                       for t in "hl"]

        # PE p-state warmup: dummy DoubleRow matmuls on a zeroed tile
        # burn the cost model's clock ramp (~3us of accumulated busy
        # before full speed) during the otherwise-idle head DMA wait.
        wut = const.tile([P, 2, 256], dt.float8e4)
        nc.vector.memset(wut[:], 0)
        for i in range(32):
            pw = p1pool.tile([P, 2, CK], dt.float32, name="p1")
            nc.tensor.matmul(
                pw[:, 0], wut[:, :, :P], wut[:], start=True, stop=True,
                perf_mode=PM.DoubleRow)

        load_x(0)
        for si, (fb0, nfb) in enumerate([(0, 2), (2, 2), (4, 4), (8, 4), (12, 4)]):
            sl = slice(fb0, fb0 + nfb)
            nc.sync.dma_start(w1_s["a"][0][:, sl], w1r["a"][0][:, sl])
            nc.sync.dma_start(w1_s["a"][1][:, sl], w1r["a"][1][:, sl])
            if si == 0:
                nc.sync.dma_start(b1_s[:], b1d.rearrange("p (s f) -> p s f", s=2))
                if NCT > 1:
                    load_x(1)
        nc.sync.dma_start(wdv_s[:], wdv[:])
        b2c_s = None
        if not fuse2:
            b2c_s = const.tile([P, 2, HBK], dt.float32)
        # x chunks that get fresh buffers interleave with the weight
        # stream; the tail chunks (recycled buffers, whose DMA waits for
        # the earlier reader) go last so the wait blocks nothing.
        nfresh = min(2 * NCT, 12) // 2
        nxt = 2
        for hb in range(0, HBK, 2):
            sl = slice(hb, hb + 2)
            nc.sync.dma_start(w2_s["a"][0][:, sl], w2r["a"][0][:, sl])
            nc.sync.dma_start(w2_s["a"][1][:, sl], w2r["a"][1][:, sl])
            if hb == 0 and not fuse2:
                nc.sync.dma_start(b2c_s[:], b2c.rearrange("p (s h) -> p s h", s=2))
        # x2/x3 after the full w2a stream: mm2(c0)'s later groups need the
        # w2a tail before mm1(c2)/mm1(c3) need these chunks.
        while nxt < min(4, nfresh):
            load_x(nxt)
            nxt += 1
        for fb0 in range(0, FB, 4):
            sl = slice(fb0, fb0 + 4)
            nc.sync.dma_start(w1_s["b"][0][:, sl], w1r["b"][0][:, sl])
            nc.sync.dma_start(w1_s["b"][1][:, sl], w1r["b"][1][:, sl])
            if nxt < nfresh:
                load_x(nxt)
                nxt += 1
        for hb in range(0, HBK, 2):
            sl = slice(hb, hb + 2)
            nc.sync.dma_start(w2_s["b"][0][:, sl], w2r["b"][0][:, sl])
            nc.sync.dma_start(w2_s["b"][1][:, sl], w2r["b"][1][:, sl])
            if nxt < nfresh:
                load_x(nxt)
                nxt += 1
        while nxt < NCT:
            load_x(nxt)
            nxt += 1

        hs = [None] * NCT

        def alloc_h(c):
            h8 = h8pool.tile([P, G, 2, CK], dt.float8e4, name="h8")
            hl = hlpool.tile([P, G, 2, CK], dt.float8e4, name="hl")
            hs[c] = (h8, hl)

        def emit_mm1_group(c, fbp):
            off, csz, seg = chunks[c]
            sk = "ab"[seg]
            w1hs, w1ls = w1_s[sk]
            xht, xlt = xh_s[c], xl_s[c]
            h8, hl = hs[c]
            ps = p1pool.tile([P, 2, CK], dt.float32, name="p1")
            for half in range(2):
                fb = 2 * fbp + half
                reg = ps[:, half, :csz]
                for j in range(J):
                    nc.tensor.matmul(
                        reg, w1hs[:, fb, j], xht[:, j, :, :csz],
                        start=(j == 0), stop=False, perf_mode=PM.DoubleRow)
                for j in range(J):
                    nc.tensor.matmul(
                        reg, w1hs[:, fb, j], xlt[:, j, :, :csz],
                        start=False, stop=False, perf_mode=PM.DoubleRow)
                for j in range(J):
                    nc.tensor.matmul(
                        reg, w1ls[:, fb, j], xht[:, j, :, :csz],
                        start=False, stop=(j == J - 1), perf_mode=PM.DoubleRow)
            g32 = gpool.tile([P, 2, CK], dt.float32, name="g32")
            h8v = h8[:, fbp, :, :csz]
            if fuse1:
                nc.scalar.activation(
                    g32[:, :, :csz], ps[:, :, :csz], AF.Gelu,
                    bias=0.0, scale=1.0 / 256)
                nc.scalar.activation(
                    h8v, ps[:, :, :csz], AF.Gelu, bias=0.0, scale=1.0 / 256)
            else:
                for half in range(2):
                    fb = 2 * fbp + half
                    nc.scalar.activation(
                        g32[:, half, :csz], ps[:, half, :csz], AF.Gelu,
                        bias=b1_s[:, seg, fb:fb + 1], scale=1.0 / 256)
                    nc.scalar.activation(
                        h8[:, fbp, half, :csz], ps[:, half, :csz], AF.Gelu,
                        bias=b1_s[:, seg, fb:fb + 1], scale=1.0 / 256)
            nc.vector.tensor_tensor(
                hl[:, fbp, :, :csz], g32[:, :, :csz], h8v, ALU.subtract)

        def emit_mm2(c):
            # Tokens ride the FREE dim (stationary w2, moving h), so mm2
            # cost is proportional to the chunk's actual token count and
            # the per-token combine weight is a plain elementwise mult.
            off, csz, seg = chunks[c]
            sk = "ab"[seg]
            w2hs, w2ls = w2_s[sk]
            h8, hl = hs[c]
            for hbp in range(HBK // 2):
                ps2 = p2pool.tile([P, 2, CK], dt.float32, name="p2")
                for half in range(2):
                    hb = 2 * hbp + half
                    reg = ps2[:, half, :csz]
                    for g in range(G):
                        nc.tensor.matmul(
                            reg, w2hs[:, hb, g], h8[:, g, :, :csz],
                            start=(g == 0), stop=False, perf_mode=PM.DoubleRow)
                    for g in range(G):
                        nc.tensor.matmul(
                            reg, w2hs[:, hb, g], hl[:, g, :, :csz],
                            start=False, stop=False, perf_mode=PM.DoubleRow)
                    for g in range(G):
                        nc.tensor.matmul(
                            reg, w2ls[:, hb, g], h8[:, g, :, :csz],
                            start=False, stop=(g == G - 1), perf_mode=PM.DoubleRow)
                ob = opool.tile([P, 2, CK], dt.float32, name="ob")
                for half in range(2):
                    hb = 2 * hbp + half
                    if fuse2:
                        nc.vector.tensor_tensor(
                            ob[:, half, :csz], ps2[:, half, :csz],
                            wdv_s[:, off:off + csz], ALU.mult)
                    else:
                        nc.vector.tensor_scalar_add(
                            ob[:, half, :csz], ps2[:, half, :csz],
                            b2c_s[:, seg, hb:hb + 1])
                        nc.vector.tensor_tensor(
                            ob[:, half, :csz], ob[:, half, :csz],
                            wdv_s[:, off:off + csz], ALU.mult)
                nc.sync.dma_start(
                    yc_r[:, 2 * hbp:2 * hbp + 2, off:off + csz],
                    ob[:, :, :csz])

        def emit_mm1(c):
            alloc_h(c)
            for fbp in range(FB // 2):
                emit_mm1_group(c, fbp)

        # Software pipeline: the two head chunks' mm1s interleave by
        # fb-pair so each arriving w1 slice feeds two PE groups (PE
        # covers the w1 DMA stream with no idle); afterwards mm1 stays
        # two chunks ahead of mm2 so the w2/w1B streams land in time.
        if NCT > 1:
            alloc_h(0)
            alloc_h(1)
            for fbp in range(FB // 2):
                emit_mm1_group(0, fbp)
                emit_mm1_group(1, fbp)
        else:
            emit_mm1(0)
        for c in range(NCT):
            emit_mm2(c)
            if c + 2 < NCT:
                emit_mm1(c + 2)
    return nc


def _get_nc(S0, S1, L0, L1, fuse1=True, fuse2=True):
    key = (S0, S1, L0, L1, fuse1, fuse2)
    if key not in _CACHE:
        nc = _build_nc(S0, S1, L0, L1, fuse1, fuse2)
        nc.finalize()
        _CACHE[key] = nc
    return _CACHE[key]


def _split8(a):
    hi = a.astype(fp8)
    lo = (a - hi.astype(np.float32)).astype(fp8)
    return hi, lo


def _x_layout(x8, chunks, idxA, idxB, S0):
    """[H, T] fp8 + chunk list -> [P, NCT*2048] in [p, c, j, i, t] layout,
    one 256-padded block per chunk in chunk-list order."""
    cols = np.zeros(len(chunks) * CK, dtype=np.int64)
    for ci, (off, csz, seg) in enumerate(chunks):
        idx = idxA if seg == 0 else idxB
        pos = off - (0 if seg == 0 else S0)
        take = idx[pos:min(pos + csz, len(idx))]
        cols[ci * CK:ci * CK + len(take)] = take
    g = x8[:, cols]                                  # [H, NCT*256]
    NCT_ = len(chunks)
    g = g.reshape(J, 2, P, NCT_, CK)                 # [j, i, p, c, t]
    return np.ascontiguousarray(
        g.transpose(2, 3, 0, 1, 4).reshape(P, NCT_ * CK * 8))


def _w1_layout(a):
    """[H, FH] -> [P, FB*8*P] as [p, fb, j, i, f]."""
    return np.ascontiguousarray(
        a.reshape(J, 2, P, FB, P).transpose(2, 3, 0, 1, 4).reshape(P, -1))


def _w2_layout(a):
    """[FH, H] -> [P, (H//128)*G*2*128] as [p, hb, g, i, h]."""
    return np.ascontiguousarray(
        a.reshape(G, 2, P, H // P, P).transpose(2, 3, 0, 1, 4).reshape(P, -1))


def dispatch(hidden_states, router_w, router_b):
    """Host router: exact fp32 softmax top-2 + renormalized weights."""
    x = np.asarray(hidden_states, dtype=np.float32).reshape(T, H)
    logits = x @ np.asarray(router_w, dtype=np.float32)
    logits = logits + np.asarray(router_b, dtype=np.float32)
    part = np.argpartition(logits, E - 2, axis=1)[:, E - 2:]     # top-2 ids
    lg = np.take_along_axis(logits, part, axis=1)                # [T, 2]
    m = lg.max(axis=1, keepdims=True)
    e = np.exp(lg - m)
    wslot = e / e.sum(axis=1, keepdims=True)                     # [T, 2]
    idx_lists, wts = [], []
    for m_ in range(E):
        hit = part == m_
        rows = np.where(hit.any(axis=1))[0]
        idx_lists.append(rows)
        wts.append((wslot * hit)[rows].sum(axis=1))
    return x, idx_lists, wts


def _pad128(n):
    return max(P, ((n + P - 1) // P) * P)


def make_in_maps(hidden_states, router_w, router_b, w1, b1, w2, b2):
    x, idx_lists, wts = dispatch(hidden_states, router_w, router_b)
    loads = np.array([len(ix) for ix in idx_lists])
    order = np.argsort(-loads, kind="stable")
    hots, colds = order[:4], order[4:]
    L0 = max(int(loads[hots].max()), 1)
    L1 = max(int(loads[colds].max()), 1)
    S0 = _pad128(L0)
    S1 = _pad128(L1)
    C = S0 + S1
    xt = np.ascontiguousarray(x.T)                   # [H, T] f32
    x8h, x8l = _split8(xt)
    w1 = np.asarray(w1, dtype=np.float32)
    w2 = np.asarray(w2, dtype=np.float32)
    b1 = np.asarray(b1, dtype=np.float32)
    b2 = np.asarray(b2, dtype=np.float32)
    fuse1 = not b1.any()
    fuse2 = not b2.any()
    pairs = list(zip(hots, colds))
    in_maps = []
    for eA, eB in pairs:
        ixA, ixB = idx_lists[eA], idx_lists[eB]
        chunks = _chunks(S0, S1, L0, L1)
        xh_full = _x_layout(x8h, chunks, ixA, ixB, S0)
        xl_full = _x_layout(x8l, chunks, ixA, ixB, S0)
        wcol = np.zeros(C, dtype=np.float32)
        wcol[:len(ixA)] = wts[eA] / 256.0
        wcol[S0:S0 + len(ixB)] = wts[eB] / 256.0
        # combine weights ride the free (token) dim: replicate across rows
        wdv_m = np.ascontiguousarray(np.broadcast_to(wcol, (P, C)))
        for side in range(2):
            fsl = slice(side * FH, (side + 1) * FH)
            im = {"xh": xh_full, "xl": xl_full, "wdv": wdv_m}
            for s, e_ in (("a", eA), ("b", eB)):
                hi1, lo1 = _split8(w1[e_][:, fsl] * 256.0)
                im[f"w1{s}h"], im[f"w1{s}l"] = _w1_layout(hi1), _w1_layout(lo1)
                hi2, lo2 = _split8(w2[e_][fsl, :] * 256.0)
                im[f"w2{s}h"], im[f"w2{s}l"] = _w2_layout(hi2), _w2_layout(lo2)
            b1m = np.stack([
                b1[eA][fsl].reshape(FB, P).T, b1[eB][fsl].reshape(FB, P).T])
            im["b1d"] = np.ascontiguousarray(
                b1m.transpose(1, 0, 2).reshape(P, 2 * FB))
            # b2 is added once per token: by side 0 only. [p, seg, hb]
            if side == 0:
                b2m = np.stack([
                    (b2[eA] * 256.0).reshape(H // P, P).T,
                    (b2[eB] * 256.0).reshape(H // P, P).T])
            else:
                b2m = np.zeros((2, P, H // P), dtype=np.float32)
            im["b2c"] = np.ascontiguousarray(
                np.asarray(b2m, dtype=np.float32).transpose(1, 0, 2)
                .reshape(P, 2 * (H // P)))
            in_maps.append(im)
    return in_maps, idx_lists, (S0, S1, L0, L1), pairs, fuse1, fuse2


def run_device(in_maps, caps, fuse1=True, fuse2=True):
    from concourse.bass_utils import run_bass_kernel_spmd

    nc = _get_nc(*caps, fuse1, fuse2)
    res = run_bass_kernel_spmd(nc, in_maps, core_ids=list(range(E)))
    return res.results


def kernel(hidden_states, router_w, router_b, w1, b1, w2, b2):
    in_maps, idx_lists, caps, pairs, fuse1, fuse2 = make_in_maps(
        hidden_states, router_w, router_b, w1, b1, w2, b2)
    S0 = caps[0]
    # One retry guards against a rare transient execution glitch observed on
    # the very first load of a freshly compiled NEFF (garbage ~1e35 values);
    # a healthy output has absmax of a few units.
    last_err = None
    acc = None
    for attempt in range(3):
        try:
            results = run_device(in_maps, caps, fuse1, fuse2)
        except Exception as e:  # transient NRT/axon failures observed
            last_err = e
            import time as _time
            _time.sleep(10)
            continue
        acc = np.zeros((T, H), dtype=np.float32)
        for i, (eA, eB) in enumerate(pairs):
            y0 = np.asarray(results[2 * i]["yc"], dtype=np.float32)
            y1 = np.asarray(results[2 * i + 1]["yc"], dtype=np.float32)
            ysum = (y0 + y1).T                       # [H, C] -> [C, H]
            ixA, ixB = idx_lists[eA], idx_lists[eB]
            acc[ixA] += ysum[:len(ixA)]
            acc[ixB] += ysum[S0:S0 + len(ixB)]
        if np.isfinite(acc).all() and np.abs(acc).max() < 1e4:
            return acc.reshape(B, S, H)
    if acc is None and last_err is not None:
        raise last_err
    return acc.reshape(B, S, H)
